# revision 2
# baseline (speedup 1.0000x reference)
"""BasicAttention Trainium2 kernel.

Reference computation (per batch b):
    q = x[b] @ Wq + bq            # [S, D]
    k = x[b] @ Wk + bk            # [S, D]
    v = x[b] @ Wv + bv            # [S, D]
    s = q @ k.T / QD              # [S, S]
    w = softmax(where(mask==0, -inf, s))
    out[b] = w @ v                # [S, D]

Sharding: 8 cores = 4 batches x 2 query-halves. Each core computes K/V for
its full batch (2048 keys) plus attention for its 1024-query half. SPMD, no
collectives. The program always treats rows [0:Sq] of its x input as the
queries; for odd cores the host rotates the key axis (and mask columns) by
Sq so their query half lands at the front — softmax and P@V are invariant
to key order.

Per-core kernel (all matmuls bf16 with fp32 PSUM accumulate):
  - cast x (f32->bf16) and mask (int32->bf16) to DRAM scratch via SWDGE DMA
  - xbar DMA-transpose x^T [E, S] and mask^T [S, Sq] straight into SBUF
  - QT[d, q] / KT[d, s] projections: weights stationary, x^T moving
  - V[s, d] natural: x^T tiles stationary, Wv moving; bv added via rank-1
    (K=1) matmul accumulation
  - scores computed TRANSPOSED: ST[ks, q] = KT-stationary @ QT-moving, so
    the softmax mask-multiply is a plain elementwise op in [ks, q] layout
    and no on-chip transpose of P is ever needed
  - exp on ACT (scale=1/QD fused), mask multiply on DVE
  - denominator: ones-column matmul with P^T stationary -> denomT [q, 1]
    directly in per-partition layout; reciprocal on DVE
  - out = (P^T.T @ V) scaled by 1/denom on PSUM eviction (ACT, per-partition
    scale), f32 out
No row-max subtraction: scores/QD are in [-0.1, 0.1] so exp is safe, and
softmax is shift-invariant, matching the reference exactly.
"""

import sys

if "/opt/trn_rl_repo" not in sys.path:
    sys.path.insert(0, "/opt/trn_rl_repo")

import numpy as np

B, S_FULL, E_DIM, QD = 4, 2048, 1024, 1024
N_CORES = 8
P = 128
INV_QD = 1.0 / 1024.0  # reference divides scores by QD=1024


def _chunks(total, step):
    out = []
    c = 0
    while c < total:
        out.append((c, min(step, total - c)))
        c += step
    return out


def build_nc(S=2048, Sq=1024, E=1024, D=1024, cast_dma_x=True, cast_dma_mask=True):
    """Build + compile the per-core Bass program."""
    from contextlib import ExitStack

    import concourse.tile as tile
    from concourse import bacc, mybir

    bf16 = mybir.dt.bfloat16
    f32 = mybir.dt.float32
    i32 = mybir.dt.int32
    AF = mybir.ActivationFunctionType
    ALU = mybir.AluOpType

    NE = E // P    # e-chunks (contraction tiles for projections)
    ND = D // P    # d-tiles
    NS = S // P    # key tiles
    NQ = Sq // P   # query tiles
    NCH = 512      # matmul moving-dim chunk (one fp32 PSUM bank)
    SLAB = 1024    # psum tile free width (2 banks)
    assert Sq <= SLAB and D <= SLAB

    nc = bacc.Bacc("TRN2", target_bir_lowering=False, debug=False)

    x_d = nc.dram_tensor("x", [S, E], f32, kind="ExternalInput").ap()
    mask_d = nc.dram_tensor("mask", [Sq, S], i32, kind="ExternalInput").ap()
    wq_d = nc.dram_tensor("Wq", [E, D], f32, kind="ExternalInput").ap()
    bq_d = nc.dram_tensor("bq", [D], f32, kind="ExternalInput").ap()
    wk_d = nc.dram_tensor("Wk", [E, D], f32, kind="ExternalInput").ap()
    bk_d = nc.dram_tensor("bk", [D], f32, kind="ExternalInput").ap()
    wv_d = nc.dram_tensor("Wv", [E, D], f32, kind="ExternalInput").ap()
    bv_d = nc.dram_tensor("bv", [D], f32, kind="ExternalInput").ap()
    out_d = nc.dram_tensor("out", [Sq, D], f32, kind="ExternalOutput").ap()

    with ExitStack() as ctx:
        tc = ctx.enter_context(tile.TileContext(nc))
        dram = ctx.enter_context(tc.tile_pool(name="dram", bufs=1, space="DRAM"))

        # ---- persistent SBUF pools ----
        const = ctx.enter_context(tc.tile_pool(name="const", bufs=1))
        qt_pool = ctx.enter_context(tc.tile_pool(name="qt", bufs=1))
        kt_pool = ctx.enter_context(tc.tile_pool(name="kt", bufs=1))
        v_pool = ctx.enter_context(tc.tile_pool(name="v", bufs=1))
        pst_pool = ctx.enter_context(tc.tile_pool(name="pst", bufs=1))
        evict = ctx.enter_context(tc.tile_pool(name="evict", bufs=3))
        maskt_pool = ctx.enter_context(tc.tile_pool(name="maskt", bufs=3))
        o_pool = ctx.enter_context(tc.tile_pool(name="o", bufs=2))
        den_pool = ctx.enter_context(tc.tile_pool(name="den", bufs=2))

        # PSUM: shared matmul pool (3 x 2 banks) + denominator pool (2 x 1 bank)
        mm_psum = ctx.enter_context(tc.tile_pool(name="mm_psum", bufs=3, space="PSUM"))
        den_psum = ctx.enter_context(tc.tile_pool(name="den_psum", bufs=2, space="PSUM"))

        # constants
        ones_row = const.tile([1, P], bf16)           # rank-1 bias lhsT
        nc.vector.memset(ones_row[0:1, :], 1.0)
        ones_col = const.tile([P, 1], bf16)           # denominator rhs
        nc.vector.memset(ones_col[:, 0:1], 1.0)
        bqk_t = const.tile([P, 2 * ND], f32, name="bqk")  # bq cols | bk cols
        nc.sync.dma_start(out=bqk_t[:, 0:ND], in_=bq_d.rearrange("(o p) -> p o", p=P))
        nc.sync.dma_start(
            out=bqk_t[:, ND : 2 * ND], in_=bk_d.rearrange("(o p) -> p o", p=P)
        )
        bv_t = const.tile([1, D], bf16)
        nc.gpsimd.dma_start(out=bv_t[0:1, :], in_=bv_d.rearrange("(a d) -> a d", a=1))

        # big persistent tensors (bf16)
        QT = qt_pool.tile([P, ND, Sq], bf16)     # QT[p, dt, q] = Q[q, dt*P+p]
        KT = kt_pool.tile([P, ND, S], bf16)      # KT[p, dt, s] = K[s, dt*P+p]
        V = v_pool.tile([P, NS, D], bf16)        # V[p, st, d] = V[st*P+p, d]
        PsT = pst_pool.tile([P, NS, Sq], bf16)   # P^T[p, kt, q]

        # ---- phase 0: dtype casts to DRAM scratch ----
        x_bf = dram.tile([S, E], bf16)
        mask_bf = dram.tile([Sq, S], bf16)
        CH = 256  # rows per cast DMA
        with nc.named_scope("cast"):
            if cast_dma_x:
                for r in range(0, S, CH):
                    nc.gpsimd.dma_start(out=x_bf[r : r + CH, :], in_=x_d[r : r + CH, :])
            else:
                with tc.tile_pool(name="xstage", bufs=3, side="right") as xs:
                    for r in range(0, S, P):
                        t32 = xs.tile([P, E], f32, tag="s32")
                        nc.sync.dma_start(out=t32[:, :], in_=x_d[r : r + P, :])
                        t16 = xs.tile([P, E], bf16, tag="s16")
                        nc.vector.tensor_copy(t16[:, :], t32[:, :])
                        nc.sync.dma_start(out=x_bf[r : r + P, :], in_=t16[:, :])
            if cast_dma_mask:
                for r in range(0, Sq, CH):
                    nc.gpsimd.dma_start(
                        out=mask_bf[r : r + CH, :], in_=mask_d[r : r + CH, :]
                    )
            else:
                with tc.tile_pool(name="mstage", bufs=3, side="right") as ms:
                    for r in range(0, Sq, P):
                        ti = ms.tile([P, S], i32, tag="mi")
                        nc.sync.dma_start(out=ti[:, :], in_=mask_d[r : r + P, :])
                        tb = ms.tile([P, S], bf16, tag="mb")
                        nc.vector.tensor_copy(tb[:, :], ti[:, :])
                        nc.sync.dma_start(out=mask_bf[r : r + P, :], in_=tb[:, :])

        # ---- phase 1: x^T into SBUF (xbar transpose), then projections ----
        with tc.tile_pool(name="xt", bufs=1, side="right") as xt_pool:
            xT = xt_pool.tile([P, NE, S], bf16)  # xT[p, e, s] = x[s, e*P+p]
            with nc.named_scope("xT"):
                for e in range(NE):
                    nc.sync.dma_start(
                        out=xT[:, e, :],
                        in_=x_bf[:, e * P : (e + 1) * P],
                        transpose=True,
                    )

            # QT and KT: weights stationary (half-D panels), x^T moving
            with tc.tile_pool(name="w", bufs=2, side="right") as w_pool:
                HD = max(D // 2, P)  # half-D panel width
                for wi, (w_src, span, dst, scope) in enumerate(
                    ((wq_d, Sq, QT, "QT"), (wk_d, S, KT, "KT"))
                ):
                    with nc.named_scope(scope):
                        for h0 in range(0, D, HD):
                            wt = w_pool.tile([P, NE, HD], bf16, tag="w")
                            nc.gpsimd.dma_start(
                                out=wt[:, :, :],
                                in_=w_src[:, h0 : h0 + HD].rearrange(
                                    "(o p) d -> p o d", p=P
                                ),
                            )
                            for dtl in range(HD // P):
                                dt = h0 // P + dtl
                                w_stat = wt[:, :, dtl * P : (dtl + 1) * P]
                                pss = []
                                for s0 in range(0, span, SLAB):
                                    sw = min(SLAB, span - s0)
                                    ps = mm_psum.tile([P, SLAB], f32, tag="mm")
                                    pss.append((s0, sw, ps))
                                for e in range(NE):
                                    for s0, sw, ps in pss:
                                        for c0, cw in _chunks(sw, NCH):
                                            nc.tensor.matmul(
                                                ps[:, c0 : c0 + cw],
                                                w_stat[:, e, :],
                                                xT[:, e, s0 + c0 : s0 + c0 + cw],
                                                start=(e == 0),
                                                stop=(e == NE - 1),
                                            )
                                # evict with per-partition bias + cast to bf16
                                bias_ap = bqk_t[:, wi * ND + dt : wi * ND + dt + 1]
                                for s0, sw, ps in pss:
                                    nc.scalar.activation(
                                        dst[:, dt, s0 : s0 + sw],
                                        ps[:, 0:sw],
                                        AF.Identity,
                                        bias=bias_ap,
                                    )

            # V natural: x^T tiles stationary, Wv moving; rank-1 bias add
            with tc.tile_pool(name="wv", bufs=1, side="right") as wv_pool:
                wv_t = wv_pool.tile([P, NE, D], bf16)
                nc.gpsimd.dma_start(
                    out=wv_t[:, :, :],
                    in_=wv_d.rearrange("(o p) d -> p o d", p=P),
                )
                with nc.named_scope("V"):
                    for st in range(NS):
                        ps = mm_psum.tile([P, SLAB], f32, tag="mm")
                        for e in range(NE):
                            for c0, cw in _chunks(D, NCH):
                                nc.tensor.matmul(
                                    ps[:, c0 : c0 + cw],
                                    xT[:, e, st * P : (st + 1) * P],
                                    wv_t[:, e, c0 : c0 + cw],
                                    start=(e == 0),
                                    stop=False,
                                )
                        for c0, cw in _chunks(D, NCH):
                            nc.tensor.matmul(
                                ps[:, c0 : c0 + cw],
                                ones_row[0:1, :],
                                bv_t[0:1, c0 : c0 + cw],
                                start=False,
                                stop=True,
                            )
                        nc.scalar.copy(V[:, st, :], ps[:, 0:D])

        # ---- phase 2: transposed scores + softmax numerator ----
        with nc.named_scope("scores"):
            for kt in range(NS):
                mt = maskt_pool.tile([P, Sq], bf16, tag="maskt")
                nc.sync.dma_start(
                    out=mt[:, :],
                    in_=mask_bf[:, kt * P : (kt + 1) * P],
                    transpose=True,
                )
                ps = mm_psum.tile([P, SLAB], f32, tag="mm")
                for dt in range(ND):
                    for c0, cw in _chunks(Sq, NCH):
                        nc.tensor.matmul(
                            ps[:, c0 : c0 + cw],
                            KT[:, dt, kt * P : (kt + 1) * P],
                            QT[:, dt, c0 : c0 + cw],
                            start=(dt == 0),
                            stop=(dt == ND - 1),
                        )
                ex = evict.tile([P, Sq], bf16, tag="exp")
                nc.scalar.activation(ex[:, :], ps[:, 0:Sq], AF.Exp, scale=INV_QD)
                nc.vector.tensor_tensor(
                    PsT[:, kt, :], ex[:, :], mt[:, :], op=ALU.mult
                )

        # ---- phase 3: denominator + P@V per query tile ----
        with nc.named_scope("pv"):
            for qt in range(NQ):
                dps = den_psum.tile([P, 1], f32, tag="den")
                ops = mm_psum.tile([P, SLAB], f32, tag="mm")
                for kt in range(NS):
                    pst_tile = PsT[:, kt, qt * P : (qt + 1) * P]
                    nc.tensor.matmul(
                        dps[:, 0:1],
                        pst_tile,
                        ones_col[:, 0:1],
                        start=(kt == 0),
                        stop=(kt == NS - 1),
                    )
                    for c0, cw in _chunks(D, NCH):
                        nc.tensor.matmul(
                            ops[:, c0 : c0 + cw],
                            pst_tile,
                            V[:, kt, c0 : c0 + cw],
                            start=(kt == 0),
                            stop=(kt == NS - 1),
                        )
                rden = den_pool.tile([P, 1], f32, tag="rden")
                nc.vector.reciprocal(rden[:, 0:1], dps[:, 0:1])
                ot = o_pool.tile([P, D], f32, tag="o")
                nc.scalar.activation(ot[:, :], ops[:, 0:D], AF.Copy, scale=rden[:, 0:1])
                nc.sync.dma_start(out=out_d[qt * P : (qt + 1) * P, :], in_=ot[:, :])

    nc.compile()
    return nc


_NC_CACHE = {}


def _get_nc(key=(2048, 1024, 1024, 1024)):
    if key not in _NC_CACHE:
        _NC_CACHE[key] = build_nc(*key)
    return _NC_CACHE[key]


def shard_inputs(x, mask, ws):
    """Build per-core input maps. Odd cores get the key axis rotated by Sq so
    their query half sits at rows [0:Sq] (softmax/PV are key-order invariant)."""
    Sq = x.shape[1] // 2
    in_maps = []
    for c in range(N_CORES):
        b, h = c // 2, c % 2
        if h == 0:
            xc = x[b]
            mc = mask[b, :Sq, :]
        else:
            xc = np.concatenate([x[b, Sq:], x[b, :Sq]], axis=0)
            mc = np.concatenate([mask[b, Sq:, Sq:], mask[b, Sq:, :Sq]], axis=1)
        in_maps.append(
            {
                "x": np.ascontiguousarray(xc),
                "mask": np.ascontiguousarray(mc),
                **ws,
            }
        )
    return in_maps


def kernel(**inputs):
    """Full-problem entry point: full unsharded inputs -> full output."""
    from concourse.bass_utils import run_bass_kernel_spmd

    x = np.asarray(inputs["x"], dtype=np.float32)
    mask = np.asarray(inputs["mask"], dtype=np.int32)
    ws = {
        k: np.ascontiguousarray(np.asarray(inputs[k], dtype=np.float32))
        for k in ("Wq", "bq", "Wk", "bk", "Wv", "bv")
    }

    nc = _get_nc()
    in_maps = shard_inputs(x, mask, ws)
    res = run_bass_kernel_spmd(nc, in_maps, core_ids=list(range(N_CORES)))

    Sq = S_FULL // 2
    out = np.empty((B, S_FULL, QD), dtype=np.float32)
    for c, r in enumerate(res.results):
        b, h = c // 2, c % 2
        out[b, h * Sq : (h + 1) * Sq, :] = r["out"]
    return out
